# revision 1
# baseline (speedup 1.0000x reference)
"""Trainium2 Bass kernel: fused bmm+decay+reduce attention scorer.

Computes, for full inputs
    self_attn  [N=16, M=100, EMB=128] f32
    self_delta [N=16, M=100, L=10000, D=4] f32
    emb_table  [L+1=10001, EMB=128] f32
    value_w    [M=100] f32
the output
    out[n, l] = sum_m value_w[m] * (sum_d self_delta[n,m,l,d]) * (emb_table[1+l] . self_attn[n,m])
of shape [16, 10000] f32 (matches the reference jnp einsum chain).

Sharding: the candidate/location axis L is split 8 ways (1250 locations per
core); every core handles all 16 batch rows for its location range.  This
keeps the dominant stream (self_delta, 32 MB/core) un-replicated and only
replicates the small attn/value tensors; the embedding table is row-sharded.

Layout: the 16*100 (batch, step) rows are flattened to r = n*100 + m and
processed in 13 partition-major tiles of 128 rows.  The delta blob is staged
host-side as [1600, LSH*4] row-major, so every DMA is a dense [128 x 20KB]
transfer whose descriptors read contiguous HBM - the fast path that keeps
all 16 SDMA engines evenly loaded (~24 GB/s each).  A vw-scaled one-hot
stationary operand vwoh[p, j] = vw[m(r)] * (n(r) == j) routes each
partition's contribution to its own output row during the m-contraction, so
mixing different n inside one tile is fine.  Per tile: PE computes
S[p,l] = emb[l].attn[r(p)] (fp32r single-pass matmul, EMB on the
contraction partitions), DVE reduces D with two pairwise adds and
multiplies by S, and the one-hot fp32r matmul accumulates all 16 output
rows in PSUM across tiles.  The raw stream goes through SWDGE
(tile-ordered completion; measured ~3us/core faster than the HWDGE rings
here); constants and the output use the scalar HWDGE ring.
The 64-row remainder tile is processed first - its half-size DMA shortens
the pipeline ramp.
"""

import numpy as np

import concourse.mybir as mybir
import concourse.tile as tile
from concourse import bacc
from concourse.bass_utils import run_bass_kernel_spmd

N, M, L, EMB, D = 16, 100, 10000, 128, 4
NCORES = 8
LSH = L // NCORES  # 1250 locations per core
R = N * M  # 1600 flattened (n, m) rows
P = 128
NTILE = (R + P - 1) // P  # 13 tiles; the last holds 64 rows
TILE_ORDER = [NTILE - 1] + list(range(NTILE - 1))  # half tile first
# matmul moving-operand chunks: all >=256 (fp32r full-rate), <=512 (PSUM bank),
# and even-sized at even offsets (fp32r ISA restriction)
CHUNKS = [(0, 418), (418, 416), (834, 416)]
NCHUNK = len(CHUNKS)
FP32 = mybir.dt.float32
FP32R = mybir.dt.float32r

_NC_CACHE = {}


def _build_nc():
    nc = bacc.Bacc(
        "TRN2", target_bir_lowering=False, debug=False, num_devices=NCORES
    )
    raw_d = nc.dram_tensor("raw", [R, LSH * D], FP32, kind="ExternalInput").ap()
    embT_d = nc.dram_tensor("embT", [EMB, LSH], FP32R, kind="ExternalInput").ap()
    attnT_d = nc.dram_tensor(
        "attnT", [EMB, NTILE * P], FP32R, kind="ExternalInput"
    ).ap()
    vwoh_d = nc.dram_tensor("vwoh", [P, NTILE * N], FP32R, kind="ExternalInput").ap()
    out_d = nc.dram_tensor("out", [N, LSH], FP32, kind="ExternalOutput").ap()

    with tile.TileContext(nc) as tc:
        with (
            tc.tile_pool(name="const", bufs=1) as cpool,
            tc.tile_pool(name="raws", bufs=6) as rpool,
            tc.tile_pool(name="work", bufs=2) as wpool,
            tc.tile_pool(name="ppool", bufs=2) as ppool,
            tc.tile_pool(name="spsum", bufs=4, space="PSUM") as spool,
            tc.tile_pool(name="apsum", bufs=1, space="PSUM") as apool,
        ):
            embT = cpool.tile([EMB, LSH], FP32R, tag="embT")
            nc.scalar.dma_start(out=embT, in_=embT_d)
            attnT = cpool.tile([EMB, NTILE * P], FP32R, tag="attnT")
            nc.scalar.dma_start(out=attnT, in_=attnT_d)
            vwoh = cpool.tile([P, NTILE * N], FP32R, tag="vwoh")
            nc.scalar.dma_start(out=vwoh, in_=vwoh_d)

            # out accumulator rows n=0..15, one PSUM bank per l-chunk
            acc = apool.tile([N, NCHUNK, 512], FP32, tag="acc")

            for ti, t in enumerate(TILE_ORDER):
                rows = min(P, R - t * P)  # 128, or 64 in the first-run tile
                raw = rpool.tile([P, LSH * D], FP32, tag="raw")
                nc.gpsimd.dma_start(
                    out=raw[:rows], in_=raw_d[t * P : t * P + rows]
                )
                rv = raw.rearrange("p (l d) -> p l d", d=D)

                # delta[p,l] = sum_d raw[p,l,d] via pairwise adds
                # (garbage rows beyond `rows` are masked by zero vwoh columns)
                a1 = wpool.tile([P, LSH, 2], FP32, tag="a1")
                nc.vector.tensor_add(out=a1, in0=rv[:, :, 0:2], in1=rv[:, :, 2:4])
                a2 = wpool.tile([P, LSH], FP32, tag="a2")
                nc.vector.tensor_add(out=a2, in0=a1[:, :, 0], in1=a1[:, :, 1])

                # S[p, l] = sum_k attn[r(p)] . emb_cand[lo+l]
                # one rotating single-bank PSUM tile per chunk, so chunk j's
                # multiply can release its bank while chunk j+1 still computes
                s_tiles = []
                for j, (c0, w) in enumerate(CHUNKS):
                    s = spool.tile([P, 512], FP32, tag="s")
                    nc.tensor.matmul(
                        s[:, :w],
                        attnT[:, t * P : (t + 1) * P],
                        embT[:, c0 : c0 + w],
                        start=True,
                        stop=True,
                    )
                    s_tiles.append(s)

                # Pt[p, l] = delta[p, l] * S[p, l], per bank-aligned chunk
                p_t = ppool.tile([P, LSH], FP32R, tag="p")
                for j, (c0, w) in enumerate(CHUNKS):
                    nc.vector.tensor_mul(
                        out=p_t[:, c0 : c0 + w],
                        in0=a2[:, c0 : c0 + w],
                        in1=s_tiles[j][:, :w],
                    )

                # acc[n, l] += sum_p vw[m(p)] * (n(p)==n) * Pt[p, l]
                for j, (c0, w) in enumerate(CHUNKS):
                    nc.tensor.matmul(
                        acc[:, j, :w],
                        vwoh[:, t * N : (t + 1) * N],
                        p_t[:, c0 : c0 + w],
                        start=(ti == 0),
                        stop=(ti == NTILE - 1),
                    )

            out_sb = cpool.tile([N, LSH], FP32, tag="out_sb")
            for j, (c0, w) in enumerate(CHUNKS):
                nc.any.tensor_copy(out=out_sb[:, c0 : c0 + w], in_=acc[:, j, :w])
            nc.scalar.dma_start(out=out_d, in_=out_sb)

    nc.compile()
    return nc


def _get_nc():
    if "nc" not in _NC_CACHE:
        _NC_CACHE["nc"] = _build_nc()
    return _NC_CACHE["nc"]


def _prep_in_maps(self_attn, self_delta, emb_table, value_w):
    self_attn = np.asarray(self_attn, dtype=np.float32)
    self_delta = np.asarray(self_delta, dtype=np.float32)
    emb_table = np.asarray(emb_table, dtype=np.float32)
    value_w = np.asarray(value_w, dtype=np.float32)

    embT_full = np.ascontiguousarray(emb_table[1 : L + 1].T)  # [EMB, L]

    # column r = n*M + m of attnT holds attn[n, m, :]; zero-pad to NTILE*P
    attnT = np.zeros((EMB, NTILE * P), dtype=np.float32)
    attnT[:, :R] = self_attn.transpose(2, 0, 1).reshape(EMB, R)

    # vwoh[p, t*N + j] = vw[m(r)] * (n(r) == j),  r = t*P + p
    vwoh = np.zeros((P, NTILE * N), dtype=np.float32)
    for t in range(NTILE):
        for p in range(min(P, R - t * P)):
            r = t * P + p
            vwoh[p, t * N + (r // M)] = value_w[r % M]

    in_maps = []
    for c in range(NCORES):
        lo = c * LSH
        raw_c = np.ascontiguousarray(
            self_delta[:, :, lo : lo + LSH, :]
        ).reshape(R, LSH * D)
        in_maps.append(
            {
                "raw": raw_c,
                "embT": np.ascontiguousarray(embT_full[:, lo : lo + LSH]),
                "attnT": attnT,
                "vwoh": vwoh,
            }
        )
    return in_maps


def _run(inputs, **spmd_kwargs):
    in_maps = _prep_in_maps(
        inputs["self_attn"], inputs["self_delta"], inputs["emb_table"], inputs["value_w"]
    )
    res = run_bass_kernel_spmd(
        _get_nc(), in_maps, core_ids=list(range(NCORES)), **spmd_kwargs
    )
    out = np.concatenate([r["out"] for r in res.results], axis=1)  # [N, L]
    return out, res


def kernel(**inputs) -> np.ndarray:
    out, _ = _run(inputs)
    return out



# revision 2
# speedup vs baseline: 1.5581x; 1.5581x over previous
"""Trainium2 Bass kernel: fused bmm+decay+reduce attention scorer.

Computes, for full inputs
    self_attn  [N=16, M=100, EMB=128] f32
    self_delta [N=16, M=100, L=10000, D=4] f32
    emb_table  [L+1=10001, EMB=128] f32
    value_w    [M=100] f32
the output
    out[n, l] = sum_m value_w[m] * (sum_d self_delta[n,m,l,d]) * (emb_table[1+l] . self_attn[n,m])
of shape [16, 10000] f32 (matches the reference jnp einsum chain).

Sharding: the candidate/location axis L is split 8 ways (1250 locations per
core); every core handles all 16 batch rows for its location range.  This
keeps the dominant stream (self_delta) un-replicated and only replicates the
small attn/value tensors; the embedding table is row-sharded.

The delta stream is staged host-side in fp16 (the 2e-2 rel-err budget dwarfs
fp16's ~1e-3) and in d-plane-major order raw[r, d*1250 + l], r = n*100 + m,
so every DVE op below runs dense step-1 16-bit operands = 2x perf mode:
  - the D-reduction is two dense pairwise adds (d0+d2, d1+d3, then fold),
  - the decay multiply is one dense fp16 mul against S staged in SBUF.
Per 128-row tile: PE computes S[p,l] = emb[l].attn[r(p)] (fp16 matmul, EMB
on the contraction partitions) into a 3-bank PSUM tile; the otherwise-idle
ScalarE evacuates S to SBUF as fp16 (freeing the DVE from 1x-rate PSUM
reads); DVE folds D and multiplies; and a vw-scaled one-hot stationary
operand vwoh[p, j] = vw[m(r)] * (n(r) == j) routes each partition's
contribution to its own output row, accumulating all 16 rows in PSUM across
tiles.  The acc matmuls are software-pipelined one tile behind the S
matmuls so the PE never idles on the S->ScalarE->mul chain.  The raw
stream goes through SWDGE (measured faster than the HWDGE rings here);
constants and the output use the scalar HWDGE ring.  The 64-row remainder
tile (zero-padded host-side) is processed first - its half-size compute
shortens the pipeline ramp.
"""

import numpy as np

import concourse.mybir as mybir
import concourse.tile as tile
from concourse import bacc
from concourse.bass_utils import run_bass_kernel_spmd

N, M, L, EMB, D = 16, 100, 10000, 128, 4
NCORES = 8
LSH = L // NCORES  # 1250 locations per core
R = N * M  # 1600 flattened (n, m) rows
P = 128
NTILE = (R + P - 1) // P  # 13 tiles; the last holds 64 real rows
RPAD = NTILE * P  # 1664 rows after zero-padding
TILE_ORDER = [NTILE - 1] + list(range(NTILE - 1))  # half tile first
# matmul moving-operand chunks: <=512 (PSUM bank), bank-aligned offsets
CHUNKS = [(0, 512), (512, 512), (1024, 226)]
FP16 = mybir.dt.float16
FP32 = mybir.dt.float32

_NC_CACHE = {}


def _build_nc():
    nc = bacc.Bacc(
        "TRN2", target_bir_lowering=False, debug=False, num_devices=NCORES
    )
    # raw[r, d*LSH + l] = self_delta[n, m, lo+l, d]  (fp16, d-plane major)
    raw_d = nc.dram_tensor("raw", [RPAD, D * LSH], FP16, kind="ExternalInput").ap()
    embT_d = nc.dram_tensor("embT", [EMB, LSH], FP16, kind="ExternalInput").ap()
    attnT_d = nc.dram_tensor("attnT", [EMB, RPAD], FP16, kind="ExternalInput").ap()
    vwoh_d = nc.dram_tensor("vwoh", [P, NTILE * N], FP16, kind="ExternalInput").ap()
    out_d = nc.dram_tensor("out", [N, LSH], FP32, kind="ExternalOutput").ap()

    with tile.TileContext(nc) as tc:
        with (
            tc.tile_pool(name="const", bufs=1) as cpool,
            tc.tile_pool(name="raws", bufs=6) as rpool,
            tc.tile_pool(name="a1p", bufs=2) as a1pool,
            tc.tile_pool(name="work", bufs=2) as wpool,
            tc.tile_pool(name="spsum", bufs=1, space="PSUM") as spool,
            tc.tile_pool(name="apsum", bufs=1, space="PSUM") as apool,
        ):
            embT = cpool.tile([EMB, LSH], FP16, tag="embT")
            nc.scalar.dma_start(out=embT, in_=embT_d)
            attnT = cpool.tile([EMB, RPAD], FP16, tag="attnT")
            nc.scalar.dma_start(out=attnT, in_=attnT_d)
            vwoh = cpool.tile([P, NTILE * N], FP16, tag="vwoh")
            nc.scalar.dma_start(out=vwoh, in_=vwoh_d)

            # out accumulator rows n=0..15, 3 PSUM banks, lives whole kernel
            acc = apool.tile([N, LSH], FP32, tag="acc")

            pending = None  # (pt, t) of the previous tile, acc-mm'd next iter

            def emit_acc(pt, t, *, first, last):
                for c0, w in CHUNKS:
                    nc.tensor.matmul(
                        acc[:, c0 : c0 + w],
                        vwoh[:, t * N : (t + 1) * N],
                        pt[:, c0 : c0 + w],
                        start=first,
                        stop=last,
                    )

            for ti, t in enumerate(TILE_ORDER):
                raw = rpool.tile([P, D * LSH], FP16, tag="raw")
                nc.gpsimd.dma_start(out=raw, in_=raw_d[t * P : (t + 1) * P])

                # S[p, l] = attn[r(p)] . emb_cand[lo+l], fp16 in, fp32 PSUM out
                s_ps = spool.tile([P, LSH], FP32, tag="s")
                for c0, w in CHUNKS:
                    nc.tensor.matmul(
                        s_ps[:, c0 : c0 + w],
                        attnT[:, t * P : (t + 1) * P],
                        embT[:, c0 : c0 + w],
                        start=True,
                        stop=True,
                    )
                # previous tile's output accumulation rides behind this
                # tile's S matmuls in the PE stream
                if pending is not None:
                    emit_acc(*pending, first=(ti == 1), last=False)

                # ScalarE evacuates S to SBUF as fp16 (2x-mode DVE operand)
                s_sb = wpool.tile([P, LSH], FP16, tag="ssb")
                nc.scalar.copy(out=s_sb, in_=s_ps)

                # delta[p,l] = sum_d raw[p,l,d]: two dense 2x-mode adds
                a1 = a1pool.tile([P, 2 * LSH], FP16, tag="a1")
                nc.vector.tensor_add(
                    out=a1, in0=raw[:, 0 : 2 * LSH], in1=raw[:, 2 * LSH : 4 * LSH]
                )
                a2 = wpool.tile([P, LSH], FP16, tag="a2")
                nc.vector.tensor_add(
                    out=a2, in0=a1[:, 0:LSH], in1=a1[:, LSH : 2 * LSH]
                )
                # Pt[p, l] = delta[p, l] * S[p, l]
                pt = wpool.tile([P, LSH], FP16, tag="pt")
                nc.vector.tensor_mul(out=pt, in0=a2, in1=s_sb)
                pending = (pt, t)

            emit_acc(*pending, first=False, last=True)

            out_sb = cpool.tile([N, LSH], FP32, tag="out_sb")
            nc.vector.tensor_copy(out=out_sb, in_=acc)
            nc.scalar.dma_start(out=out_d, in_=out_sb)

    nc.compile()
    return nc


def _get_nc():
    if "nc" not in _NC_CACHE:
        _NC_CACHE["nc"] = _build_nc()
    return _NC_CACHE["nc"]


def _prep_in_maps(self_attn, self_delta, emb_table, value_w):
    self_attn = np.asarray(self_attn, dtype=np.float32)
    self_delta = np.asarray(self_delta, dtype=np.float32)
    emb_table = np.asarray(emb_table, dtype=np.float32)
    value_w = np.asarray(value_w, dtype=np.float32)

    embT_full = emb_table[1 : L + 1].T.astype(np.float16)  # [EMB, L]

    # column r = n*M + m of attnT holds attn[n, m, :]; zero-pad to RPAD
    attnT = np.zeros((EMB, RPAD), dtype=np.float16)
    attnT[:, :R] = self_attn.transpose(2, 0, 1).reshape(EMB, R)

    # vwoh[p, t*N + j] = vw[m(r)] * (n(r) == j),  r = t*P + p
    vwoh = np.zeros((P, NTILE * N), dtype=np.float16)
    for t in range(NTILE):
        for p in range(min(P, R - t * P)):
            r = t * P + p
            vwoh[p, t * N + (r // M)] = value_w[r % M]

    in_maps = []
    for c in range(NCORES):
        lo = c * LSH
        # raw[r, d*LSH + l] = self_delta[n, m, lo+l, d], fp16, rows padded
        raw_c = np.zeros((RPAD, D * LSH), dtype=np.float16)
        raw_c[:R].reshape(N, M, D, LSH)[...] = self_delta[
            :, :, lo : lo + LSH, :
        ].transpose(0, 1, 3, 2)
        in_maps.append(
            {
                "raw": raw_c,
                "embT": np.ascontiguousarray(embT_full[:, lo : lo + LSH]),
                "attnT": attnT,
                "vwoh": vwoh,
            }
        )
    return in_maps


def _run(inputs, **spmd_kwargs):
    in_maps = _prep_in_maps(
        inputs["self_attn"], inputs["self_delta"], inputs["emb_table"], inputs["value_w"]
    )
    res = run_bass_kernel_spmd(
        _get_nc(), in_maps, core_ids=list(range(NCORES)), **spmd_kwargs
    )
    out = np.concatenate([r["out"] for r in res.results], axis=1)  # [N, L]
    return out, res


def kernel(**inputs) -> np.ndarray:
    out, _ = _run(inputs)
    return out


# revision 4
# speedup vs baseline: 1.7073x; 1.0958x over previous
"""Trainium2 Bass kernel: fused bmm+decay+reduce attention scorer.

Computes, for full inputs
    self_attn  [N=16, M=100, EMB=128] f32
    self_delta [N=16, M=100, L=10000, D=4] f32
    emb_table  [L+1=10001, EMB=128] f32
    value_w    [M=100] f32
the output
    out[n, l] = sum_m value_w[m] * (sum_d self_delta[n,m,l,d]) * (emb_table[1+l] . self_attn[n,m])
of shape [16, 10000] f32 (matches the reference jnp einsum chain).

Sharding: the candidate/location axis L is split 8 ways (1250 locations per
core); every core handles all 16 batch rows for its location range.  This
keeps the dominant stream (self_delta) un-replicated and only replicates the
small attn/value tensors; the embedding table is row-sharded.

The delta stream is staged host-side in fp16 (the 2e-2 rel-err budget dwarfs
fp16's ~1e-3) and in d-plane-major order raw[r, d*1250 + l], r = n*100 + m,
so every DVE op below runs dense step-1 16-bit operands = 2x perf mode:
  - the D-reduction is two dense pairwise adds (d0+d2, d1+d3, then fold),
  - the decay multiply is one dense fp16 mul against S staged in SBUF.
Per 128-row tile: PE computes S[p,l] = emb[l].attn[r(p)] (fp16 matmul, EMB
on the contraction partitions) into a 3-bank PSUM tile; the otherwise-idle
ScalarE evacuates S to SBUF as fp16 (freeing the DVE from 1x-rate PSUM
reads); DVE folds D and multiplies; and a vw-scaled one-hot stationary
operand vwoh[p, j] = vw[m(r)] * (n(r) == j) routes each partition's
contribution to its own output row, accumulating all 16 rows in PSUM across
tiles.  The acc matmuls are software-pipelined one tile behind the S
matmuls so the PE never idles on the S->ScalarE->mul chain.  The raw
stream goes through SWDGE (measured faster than the HWDGE rings here);
constants and the output use the scalar HWDGE ring.  The 64-row remainder
tile (zero-padded host-side) is processed first - its half-size compute
shortens the pipeline ramp.
"""

import ml_dtypes
import numpy as np

import concourse.mybir as mybir
import concourse.tile as tile
from concourse import bacc
from concourse.bass_utils import run_bass_kernel_spmd

BF16 = ml_dtypes.bfloat16

N, M, L, EMB, D = 16, 100, 10000, 128, 4
NCORES = 8
LSH = L // NCORES  # 1250 locations per core
R = N * M  # 1600 flattened (n, m) rows
P = 128
NTILE = (R + P - 1) // P  # 13 tiles; the last holds 64 real rows
RPAD = NTILE * P  # 1664 rows after zero-padding
TILE_ORDER = [NTILE - 1] + list(range(NTILE - 1))  # half tile first
# matmul moving-operand chunks: <=512 (PSUM bank), bank-aligned offsets
CHUNKS = [(0, 512), (512, 512), (1024, 226)]
FP16 = mybir.dt.bfloat16
FP32 = mybir.dt.float32

_NC_CACHE = {}


def _build_nc():
    nc = bacc.Bacc(
        "TRN2", target_bir_lowering=False, debug=False, num_devices=NCORES
    )
    # raw[r, d*LSH + l] = self_delta[n, m, lo+l, d]  (fp16, d-plane major)
    raw_d = nc.dram_tensor("raw", [RPAD, D * LSH], FP16, kind="ExternalInput").ap()
    embT_d = nc.dram_tensor("embT", [EMB, LSH], FP16, kind="ExternalInput").ap()
    attnT_d = nc.dram_tensor("attnT", [EMB, RPAD], FP16, kind="ExternalInput").ap()
    vwoh_d = nc.dram_tensor("vwoh", [P, NTILE * N], FP16, kind="ExternalInput").ap()
    out_d = nc.dram_tensor("out", [N, LSH], FP32, kind="ExternalOutput").ap()

    with tile.TileContext(nc) as tc:
        with (
            tc.tile_pool(name="const", bufs=1) as cpool,
            tc.tile_pool(name="raws", bufs=8) as rpool,
            tc.tile_pool(name="a1p", bufs=2) as a1pool,
            tc.tile_pool(name="work", bufs=2) as wpool,
            tc.tile_pool(name="spsum", bufs=1, space="PSUM") as spool,
            tc.tile_pool(name="apsum", bufs=1, space="PSUM") as apool,
        ):
            embT = cpool.tile([EMB, LSH], FP16, tag="embT")
            nc.scalar.dma_start(out=embT, in_=embT_d)
            attnT = cpool.tile([EMB, RPAD], FP16, tag="attnT")
            nc.scalar.dma_start(out=attnT, in_=attnT_d)
            vwoh = cpool.tile([P, NTILE * N], FP16, tag="vwoh")
            nc.scalar.dma_start(out=vwoh, in_=vwoh_d)

            # out accumulator rows n=0..15, 3 PSUM banks, lives whole kernel
            acc = apool.tile([N, LSH], FP32, tag="acc")

            pending = None  # (pt, t) of the previous tile, acc-mm'd next iter

            def emit_acc(pt, t, *, first, last):
                nc.tensor.ldweights(vwoh[:, t * N : (t + 1) * N])
                for c0, w in CHUNKS:
                    nc.tensor.matmul(
                        acc[:, c0 : c0 + w],
                        vwoh[:, t * N : (t + 1) * N],
                        pt[:, c0 : c0 + w],
                        start=first,
                        stop=last,
                    )

            for ti, t in enumerate(TILE_ORDER):
                raw = rpool.tile([P, D * LSH], FP16, tag="raw")
                nc.gpsimd.dma_start(out=raw, in_=raw_d[t * P : (t + 1) * P])

                # S[p, l] = attn[r(p)] . emb_cand[lo+l], fp16 in, fp32 PSUM out
                s_ps = spool.tile([P, LSH], FP32, tag="s")
                nc.tensor.ldweights(attnT[:, t * P : (t + 1) * P])
                for c0, w in CHUNKS:
                    nc.tensor.matmul(
                        s_ps[:, c0 : c0 + w],
                        attnT[:, t * P : (t + 1) * P],
                        embT[:, c0 : c0 + w],
                        start=True,
                        stop=True,
                    )
                # previous tile's output accumulation rides behind this
                # tile's S matmuls in the PE stream
                if pending is not None:
                    emit_acc(*pending, first=(ti == 1), last=False)

                # ScalarE evacuates S to SBUF as fp16 (2x-mode DVE operand)
                s_sb = wpool.tile([P, LSH], FP16, tag="ssb")
                nc.scalar.copy(out=s_sb, in_=s_ps)

                # delta[p,l] = sum_d raw[p,l,d]: two dense 2x-mode adds
                a1 = a1pool.tile([P, 2 * LSH], FP16, tag="a1")
                nc.vector.tensor_add(
                    out=a1, in0=raw[:, 0 : 2 * LSH], in1=raw[:, 2 * LSH : 4 * LSH]
                )
                a2 = wpool.tile([P, LSH], FP16, tag="a2")
                nc.vector.tensor_add(
                    out=a2, in0=a1[:, 0:LSH], in1=a1[:, LSH : 2 * LSH]
                )
                # Pt[p, l] = delta[p, l] * S[p, l]
                pt = wpool.tile([P, LSH], FP16, tag="pt")
                nc.vector.tensor_mul(out=pt, in0=a2, in1=s_sb)
                pending = (pt, t)

            emit_acc(*pending, first=False, last=True)

            out_sb = cpool.tile([N, LSH], FP32, tag="out_sb")
            nc.scalar.copy(out=out_sb, in_=acc)
            nc.scalar.dma_start(out=out_d, in_=out_sb)

    nc.compile()
    return nc


def _get_nc():
    if "nc" not in _NC_CACHE:
        _NC_CACHE["nc"] = _build_nc()
    return _NC_CACHE["nc"]


def _prep_in_maps(self_attn, self_delta, emb_table, value_w):
    self_attn = np.asarray(self_attn, dtype=np.float32)
    self_delta = np.asarray(self_delta, dtype=np.float32)
    emb_table = np.asarray(emb_table, dtype=np.float32)
    value_w = np.asarray(value_w, dtype=np.float32)

    embT_full = emb_table[1 : L + 1].T.astype(BF16)  # [EMB, L]

    # column r = n*M + m of attnT holds attn[n, m, :]; zero-pad to RPAD
    attnT = np.zeros((EMB, RPAD), dtype=BF16)
    attnT[:, :R] = self_attn.transpose(2, 0, 1).reshape(EMB, R)

    # vwoh[p, t*N + j] = vw[m(r)] * (n(r) == j),  r = t*P + p
    vwoh = np.zeros((P, NTILE * N), dtype=BF16)
    for t in range(NTILE):
        for p in range(min(P, R - t * P)):
            r = t * P + p
            vwoh[p, t * N + (r // M)] = value_w[r % M]

    in_maps = []
    for c in range(NCORES):
        lo = c * LSH
        # raw[r, d*LSH + l] = self_delta[n, m, lo+l, d], fp16, rows padded
        raw_c = np.zeros((RPAD, D * LSH), dtype=BF16)
        raw_c[:R].reshape(N, M, D, LSH)[...] = self_delta[
            :, :, lo : lo + LSH, :
        ].transpose(0, 1, 3, 2)
        in_maps.append(
            {
                "raw": raw_c,
                "embT": np.ascontiguousarray(embT_full[:, lo : lo + LSH]),
                "attnT": attnT,
                "vwoh": vwoh,
            }
        )
    return in_maps


def _run(inputs, **spmd_kwargs):
    in_maps = _prep_in_maps(
        inputs["self_attn"], inputs["self_delta"], inputs["emb_table"], inputs["value_w"]
    )
    res = run_bass_kernel_spmd(
        _get_nc(), in_maps, core_ids=list(range(NCORES)), **spmd_kwargs
    )
    out = np.concatenate([r["out"] for r in res.results], axis=1)  # [N, L]
    return out, res


def kernel(**inputs) -> np.ndarray:
    out, _ = _run(inputs)
    return out
